# revision 25
# baseline (speedup 1.0000x reference)
"""BlockSparseMLP (MoE top-2 routing, 8 experts) — Trainium2 Bass kernel.

Strategy v2 (expert-group tensor-parallel, "TP-4"):

The 8 experts are split into 2 groups of 4, chosen to balance total token
load across groups (sorted-alternating assignment).  Each group owns 4
cores; core (g, q) holds the q-th F-quarter (1408 = 11 f-tiles) of all 4
experts in group g and computes, for every token routed to those experts,
the gated-MLP over its quarter:

   gT = Wg[e][:, q].T @ xT      (bf16 matmuls, fp32 PSUM)
   uT = Wu[e][:, q].T @ xT
   aT = silu(gT) * uT
   dT_partial = Wd[e][q, :].T @ aT      (partial over the F-quarter)
   outT += dT_partial * w_e             (combine weight folded on-chip)

Host sums the 4 partial outputs of each group (the tp_reduce) and
scatter-adds into the final [T, D] output.

Why: with one-expert-per-core SPMD, every core pays the *maximum* expert
load (545 tokens for seed 0).  With 4-way F-parallelism over expert
groups, a core processes its group's token total over a quarter of F;
balanced groups bring the per-core work to ~2064 token-quarters = 516
token-equivalents, recovering the load-imbalance loss and cutting the
instruction count (11 f-tiles per phase-1 pass instead of 44).

All inputs are cast to bf16 on the host and uploaded as bf16 — HBM
traffic halves vs fp32+SWDGE-cast, which removes the startup weight-
stream starvation the fp32 version showed in traces.
"""

import os

import numpy as np

T, D, F, E, TOPK = 2048, 2048, 5632, 8, 2
P = 128
KD = D // P          # 16 k-subtiles over D
NQ = 4               # cores per expert group (F quartering)
FQ = F // NQ         # 1408 F-columns per core
KFQ = FQ // P        # 11 f-tiles per core
NFB = 3              # phase-1 weight DMA blocks per (expert, matrix)
FBW = 512            # f-columns per block (last block: 384 real + 128 pad)
FT_PER_FB = (4, 4, 3)
NDG = 8              # phase-2 d-groups (256 D-columns each)
DG = 2               # d-tiles per d-group

_COMPILED = {}       # (slot_sizes, chunks) -> nc
LAST_RESULT = None   # BassKernelResults of the most recent run (for test.py)


def _slot_chunks(s):
    """Split slot size s into <=512 even-sized chunks."""
    nch = -(-s // 512)
    base = s // nch
    sizes = [base] * nch
    for i in range(s - base * nch):
        sizes[i] += 1
    # make sizes even where possible (s is even)
    for i in range(0, len(sizes) - 1):
        if sizes[i] % 2:
            sizes[i] += 1
            sizes[i + 1] -= 1
    return sizes


def _build(slot_sizes):
    """Build + compile the SPMD Tile program for per-slot token counts."""
    import concourse.bass as bass  # noqa: F401
    import concourse.mybir as mybir
    import concourse.tile as tile
    from concourse import bacc

    f32 = mybir.dt.float32
    bf16 = mybir.dt.bfloat16
    mult = mybir.AluOpType.mult

    nslot = len(slot_sizes)
    slotmax = max(slot_sizes)
    cap = sum(slot_sizes)
    soffs = [sum(slot_sizes[:j]) for j in range(nslot)]
    chunks = [_slot_chunks(s) for s in slot_sizes]
    nch_max = max(len(c) for c in chunks)
    psum_bufs = 2 if nch_max <= 2 else 1

    nc = bacc.Bacc("TRN2", target_bir_lowering=False, debug=False,
                   enable_asserts=False, num_devices=E)

    # token tensor: per-(slot, chunk) contiguous [P, KD*cn] blocks packed
    # along the flat axis, so every segment DMA reads KD*cn-byte runs
    xt_d = nc.dram_tensor("xt", [P, KD * cap], bf16,
                          kind="ExternalInput").ap()
    wg_d = nc.dram_tensor("wg", [nslot, NFB, P, KD, FBW], bf16,
                          kind="ExternalInput").ap()
    wu_d = nc.dram_tensor("wu", [nslot, NFB, P, KD, FBW], bf16,
                          kind="ExternalInput").ap()
    # slot-0 first block again, f-slice-major: contiguous 4KB/partition
    # descriptors for the startup-critical fine-grained slices
    wgf_d = nc.dram_tensor("wgf", [4, P, KD, P], bf16,
                           kind="ExternalInput").ap()
    wuf_d = nc.dram_tensor("wuf", [4, P, KD, P], bf16,
                           kind="ExternalInput").ap()
    wd_d = nc.dram_tensor("wd", [nslot, NDG, P, KFQ, P * DG], bf16,
                          kind="ExternalInput").ap()
    wr_d = nc.dram_tensor("wrep", [P, cap], f32, kind="ExternalInput").ap()
    out_d = nc.dram_tensor("out_t", [D, cap], f32, kind="ExternalOutput").ap()
    scr_d = nc.dram_tensor("scr", [P, 512], f32).ap()   # warm-up sink

    with tile.TileContext(nc) as tc:
        with (
            tc.tile_pool(name="resident", bufs=1) as rpool,
            tc.tile_pool(name="xtp", bufs=2) as xtpool,
            tc.tile_pool(name="atp", bufs=2) as atpool,
            tc.tile_pool(name="w1", bufs=2) as w1pool,
            tc.tile_pool(name="wd2", bufs=8) as wd2pool,
            tc.tile_pool(name="outp", bufs=4) as outpool,
            tc.tile_pool(name="psum", bufs=psum_bufs, space="PSUM") as ppool,
        ):
            # ---- one SWDGE ring (gpsimd), loaded in consumption order:
            # slot-0 tokens + first weight block (f-major fine slices),
            # then per slot: w1 blocks / next-slot tokens / wd blocks.
            # sync (HWDGE) carries wrep, warm-up sink, output stores.
            xsegs = [None] * nslot

            def xseg_piece(j, c0j, cn, eng=None):
                f0 = (soffs[j] + c0j) * KD
                (eng or nc.gpsimd).dma_start(xsegs[j][:, :, c0j:c0j + cn],
                                             xt_d[:, f0:f0 + cn * KD])

            def xseg_dma(j):
                xsegs[j] = xtpool.tile([P, KD, slotmax], bf16, tag="xs",
                                       name=f"xseg_{j}")
                c0j = 0
                for cn in chunks[j]:
                    xseg_piece(j, c0j, cn)
                    c0j += cn

            w1tiles = [[None] * NFB for _ in range(nslot)]
            wgb0 = w1pool.tile([P, KD, FBW], bf16, tag="wg", name="wgb_0_0")
            wub0 = w1pool.tile([P, KD, FBW], bf16, tag="wu", name="wub_0_0")
            w1tiles[0][0] = (wgb0, wub0)
            # startup-critical pieces go on the sync HWDGE ring (fires
            # ~1us earlier than SWDGE and pulls in parallel with the
            # gpsimd weight ring)
            xsegs[0] = xtpool.tile([P, KD, slotmax], bf16, tag="xs",
                                   name="xseg_0")
            xseg_piece(0, 0, chunks[0][0], eng=nc.sync)
            c0rest = chunks[0][0]
            for s in range(4):
                sl = slice(s * P, (s + 1) * P)
                nc.sync.dma_start(wgb0[:, :, sl], wgf_d[s])
                nc.sync.dma_start(wub0[:, :, sl], wuf_d[s])
                if s == 0:
                    for cn in chunks[0][1:]:
                        xseg_piece(0, c0rest, cn, eng=nc.sync)
                        c0rest += cn

            wrep = rpool.tile([P, cap], f32)
            nc.sync.dma_start(wrep[:], wr_d)

            # Warm-up: run throwaway matmuls on a zeroed tile while the
            # first weight/token DMAs are in flight, so the PE HAM
            # clock-gate opens (1.2 -> 2.4 GHz) before real work arrives.
            warm = rpool.tile([P, 512], bf16)
            nc.vector.memset(warm[:], 0.0)
            wps = ppool.tile([P, 512], f32, tag="g0", name="warm_ps")
            for i in range(14):
                nc.tensor.matmul(wps[:], warm[:, :P], warm[:],
                                 start=(i == 0), stop=(i == 13))
            wout = rpool.tile([P, 512], f32)
            nc.vector.tensor_copy(out=wout[:], in_=wps[:])
            nc.sync.dma_start(scr_d[:], wout[:])

            at_tiles = [None] * nslot

            def phase1(j):
                jchunks = chunks[j]
                jstarts = [sum(jchunks[:i]) for i in range(len(jchunks))]
                at = atpool.tile([P, KFQ, slotmax], bf16, tag="at",
                                 name=f"at_{j}")
                at_tiles[j] = at

                # ---- phase 1: gT/uT = W.T @ xT, aT = silu(gT)*uT ----
                xseg = xsegs[j]
                for fb in range(NFB):
                    if w1tiles[j][fb] is None:
                        wgb = w1pool.tile([P, KD, FBW], bf16, tag="wg",
                                          name=f"wgb_{j}_{fb}")
                        wub = w1pool.tile([P, KD, FBW], bf16, tag="wu",
                                          name=f"wub_{j}_{fb}")
                        w1tiles[j][fb] = (wgb, wub)
                        nc.gpsimd.dma_start(wgb[:], wg_d[j, fb])
                        nc.gpsimd.dma_start(wub[:], wu_d[j, fb])
                    if fb == 1 and j + 1 < nslot:
                        xseg_dma(j + 1)
                    wgb, wub = w1tiles[j][fb]
                    for fs in range(FT_PER_FB[fb]):
                        ft = fb * 4 + fs
                        for ci, (c0, cn) in enumerate(zip(jstarts, jchunks)):
                            pg = ppool.tile([P, 512], f32, tag=f"g{ci}")
                            pu = ppool.tile([P, 512], f32, tag=f"u{ci}")
                            for ko in range(KD):
                                nc.tensor.matmul(
                                    pg[:, :cn],
                                    wgb[:, ko, fs * P:(fs + 1) * P],
                                    xseg[:, ko, c0:c0 + cn],
                                    start=(ko == 0), stop=(ko == KD - 1))
                            for ko in range(KD):
                                nc.tensor.matmul(
                                    pu[:, :cn],
                                    wub[:, ko, fs * P:(fs + 1) * P],
                                    xseg[:, ko, c0:c0 + cn],
                                    start=(ko == 0), stop=(ko == KD - 1))
                            a_sl = at[:, ft, c0:c0 + cn]
                            nc.scalar.activation(
                                a_sl, pg[:, :cn],
                                mybir.ActivationFunctionType.Silu)
                            nc.vector.tensor_tensor(a_sl, a_sl, pu[:, :cn],
                                                    mult)

            def phase2(j):
                # ---- phase 2: dT = Wd.T @ aT (partial over F-quarter),
                #      out = dT * w ----
                soff = soffs[j]
                jchunks = chunks[j]
                jstarts = [sum(jchunks[:i]) for i in range(len(jchunks))]
                at = at_tiles[j]
                wdbs = []
                for dg in range(NDG):
                    wdb = wd2pool.tile([P, KFQ, P * DG], bf16, tag="wd",
                                       name=f"wdb_{j}_{dg}")
                    nc.gpsimd.dma_start(wdb[:], wd_d[j, dg])
                    wdbs.append(wdb)
                # one pass per chunk, 2 PSUM groups per d-group, with
                # dg-parity tag rotation: a d-group's first matmul only
                # reuses a PSUM bank 4 d-groups back, so the combine
                # (DVE) latency never stalls the PE
                for ci, (c0, cn) in enumerate(zip(jstarts, jchunks)):
                    for dg in range(NDG):
                        wdb = wdbs[dg]
                        pds = [ppool.tile([P, 512], f32,
                                          tag=f"{'gu'[ds]}{dg % 2}",
                                          name=f"pd_{j}_{dg}_{ds}_{ci}")
                               for ds in range(DG)]
                        for fk in range(KFQ):
                            for ds in range(DG):
                                nc.tensor.matmul(
                                    pds[ds][:, :cn],
                                    wdb[:, fk, ds * P:(ds + 1) * P],
                                    at[:, fk, c0:c0 + cn],
                                    start=(fk == 0), stop=(fk == KFQ - 1))
                        for ds in range(DG):
                            dt_idx = dg * DG + ds
                            ot = outpool.tile([P, 512], f32, tag="ot")
                            nc.vector.tensor_tensor(
                                ot[:, :cn], pds[ds][:, :cn],
                                wrep[:, soff + c0:soff + c0 + cn], mult)
                            nc.sync.dma_start(
                                out_d[dt_idx * P:(dt_idx + 1) * P,
                                      soff + c0:soff + c0 + cn],
                                ot[:, :cn])

            # Software-pipeline the slots: each phase 2 runs one slot
            # behind its phase 1, so the trailing silu/mult of slot j's
            # last f-tile completes long before phase 2 reads it (no PE
            # stall on the DVE at phase boundaries).
            phase1(0)
            for j in range(1, nslot):
                phase1(j)
                phase2(j - 1)
            phase2(nslot - 1)

    nc.compile()
    return nc


def _swizzle_w1(wq):
    """[D, FQ] bf16 -> [NFB, P, KD, FBW] block-major (last block padded)."""
    wp = np.zeros((D, NFB * FBW), dtype=wq.dtype)
    wp[:, :FQ] = wq
    return np.ascontiguousarray(
        wp.reshape(KD, P, NFB, FBW).transpose(2, 1, 0, 3))


def _swizzle_wd(wq):
    """[FQ, D] bf16 -> [NDG, P, KFQ, P*DG] block-major."""
    return np.ascontiguousarray(
        wq.reshape(KFQ, P, NDG, P * DG).transpose(2, 1, 0, 3))


def _swizzle_w1_first(wq):
    """[D, FQ] bf16 -> [4, P, KD, P] f-slice-major (first 512 f-cols)."""
    return np.ascontiguousarray(
        wq[:, :FBW].reshape(KD, P, 4, P).transpose(2, 1, 0, 3))


def kernel(x, gate_tensor, Wg, Wu, Wd):
    global LAST_RESULT
    import ml_dtypes
    from concourse.bass_interp import get_hw_module
    from concourse.bass_utils import run_bass_kernel_spmd

    bf = ml_dtypes.bfloat16
    x = np.ascontiguousarray(np.asarray(x, dtype=np.float32))
    gate_tensor = np.asarray(gate_tensor, dtype=np.float32)
    Wg = np.asarray(Wg, dtype=np.float32)
    Wu = np.asarray(Wu, dtype=np.float32)
    Wd = np.asarray(Wd, dtype=np.float32)

    # ---- router (replicated; tiny: T*D*E flops) ----
    logits = x @ gate_tensor                      # [T, E] fp32
    m = logits.max(axis=-1, keepdims=True)
    p = np.exp(logits - m, dtype=np.float32)
    p /= p.sum(axis=-1, keepdims=True)
    topi = np.argsort(-p, axis=-1, kind="stable")[:, :TOPK]      # [T, K]
    topw = np.take_along_axis(p, topi, axis=-1)
    topw = topw / (topw.sum(axis=-1, keepdims=True) + 1e-20)

    idx = []          # tokens routed to each expert
    wts = []          # their combine weights
    for e in range(E):
        sel = (topi == e)                         # [T, K]; <=1 True per row
        idx.append(np.nonzero(sel.any(axis=-1))[0])
        wts.append(topw[sel].astype(np.float32))  # row-major == token order

    # ---- balanced expert grouping: sort by load desc, alternate ranks ----
    order = sorted(range(E), key=lambda e: -len(idx[e]))
    groups = [[order[2 * j + g] for j in range(NQ)] for g in range(2)]
    slot_sizes = tuple(
        (max(len(idx[groups[0][j]]), len(idx[groups[1][j]])) + 1) // 2 * 2
        for j in range(NQ))
    soffs = [sum(slot_sizes[:j]) for j in range(NQ)]
    cap = sum(slot_sizes)

    if slot_sizes not in _COMPILED:
        _COMPILED[slot_sizes] = _build(slot_sizes)
    nc = _COMPILED[slot_sizes]

    # ---- dispatch: per-core inputs (bf16, pre-swizzled) ----
    x_bf = x.astype(bf)
    Wg_bf = Wg.astype(bf)
    Wu_bf = Wu.astype(bf)
    Wd_bf = Wd.astype(bf)

    in_maps = []
    for g in range(2):
        # tokens + combine weights shared by the group's 4 cores;
        # xt is packed as contiguous [P, KD*cn] blocks per (slot, chunk)
        xt = np.zeros((P, KD * cap), dtype=bf)
        wr = np.zeros((P, cap), dtype=np.float32)
        for j in range(NQ):
            e = groups[g][j]
            n = len(idx[e])
            xg = x_bf[idx[e]]                      # [n, D]
            wr[:, soffs[j]:soffs[j] + n] = wts[e][None, :]
            c0 = 0
            for cn in _slot_chunks(slot_sizes[j]):
                m = max(0, min(cn, n - c0))
                blk = np.zeros((P, KD, cn), dtype=bf)
                if m:
                    blk[:, :, :m] = (
                        xg[c0:c0 + m].T.reshape(KD, P, m).transpose(1, 0, 2))
                off = (soffs[j] + c0) * KD
                xt[:, off:off + KD * cn] = blk.reshape(P, KD * cn)
                c0 += cn
        for q in range(NQ):
            fsl = slice(q * FQ, (q + 1) * FQ)
            wg = np.stack([_swizzle_w1(Wg_bf[groups[g][j]][:, fsl])
                           for j in range(NQ)])
            wu = np.stack([_swizzle_w1(Wu_bf[groups[g][j]][:, fsl])
                           for j in range(NQ)])
            wd = np.stack([_swizzle_wd(Wd_bf[groups[g][j]][fsl, :])
                           for j in range(NQ)])
            wgf = _swizzle_w1_first(Wg_bf[groups[g][0]][:, fsl])
            wuf = _swizzle_w1_first(Wu_bf[groups[g][0]][:, fsl])
            in_maps.append({"xt": xt, "wg": wg, "wu": wu, "wd": wd,
                            "wgf": wgf, "wuf": wuf, "wrep": wr})

    trace = bool(int(os.environ.get("KERNEL_TRACE", "0")))
    old_m = nc.m
    nc.m = get_hw_module(nc.m)
    try:
        try:
            res = run_bass_kernel_spmd(nc, in_maps, core_ids=list(range(E)),
                                       trace=trace)
        except (ImportError, ModuleNotFoundError):
            # tracing requested (e.g. BASS_TRACE in the env) but this image
            # lacks the axon NTFF profile hook -- rerun without tracing
            os.environ["BASS_NEVER_TRACE"] = "1"
            res = run_bass_kernel_spmd(nc, in_maps, core_ids=list(range(E)),
                                       trace=False)
    finally:
        nc.m = old_m
    LAST_RESULT = res

    # ---- combine: tp_reduce over the 4 quarters, then scatter-add ----
    out = np.zeros((T, D), dtype=np.float32)
    for g in range(2):
        acc = res.results[g * NQ]["out_t"].astype(np.float64)
        for q in range(1, NQ):
            acc += res.results[g * NQ + q]["out_t"]
        acc = acc.astype(np.float32)
        for j in range(NQ):
            e = groups[g][j]
            n = len(idx[e])
            out[idx[e]] += acc[:, soffs[j]:soffs[j] + n].T
    return out


# revision 27
# speedup vs baseline: 1.0382x; 1.0382x over previous
"""BlockSparseMLP (MoE top-2 routing, 8 experts) — Trainium2 Bass kernel.

Strategy v2 (expert-group tensor-parallel, "TP-4"):

The 8 experts are split into 2 groups of 4, chosen to balance total token
load across groups (sorted-alternating assignment).  Each group owns 4
cores; core (g, q) holds the q-th F-quarter (1408 = 11 f-tiles) of all 4
experts in group g and computes, for every token routed to those experts,
the gated-MLP over its quarter:

   gT = Wg[e][:, q].T @ xT      (bf16 matmuls, fp32 PSUM)
   uT = Wu[e][:, q].T @ xT
   aT = silu(gT) * uT
   dT_partial = Wd[e][q, :].T @ aT      (partial over the F-quarter)
   outT += dT_partial * w_e             (combine weight folded on-chip)

Host sums the 4 partial outputs of each group (the tp_reduce) and
scatter-adds into the final [T, D] output.

Why: with one-expert-per-core SPMD, every core pays the *maximum* expert
load (545 tokens for seed 0).  With 4-way F-parallelism over expert
groups, a core processes its group's token total over a quarter of F;
balanced groups bring the per-core work to ~2064 token-quarters = 516
token-equivalents, recovering the load-imbalance loss and cutting the
instruction count (11 f-tiles per phase-1 pass instead of 44).

All inputs are cast to bf16 on the host and uploaded as bf16 — HBM
traffic halves vs fp32+SWDGE-cast, which removes the startup weight-
stream starvation the fp32 version showed in traces.
"""

import os

import numpy as np

T, D, F, E, TOPK = 2048, 2048, 5632, 8, 2
P = 128
KD = D // P          # 16 k-subtiles over D
NQ = 4               # cores per expert group (F quartering)
FQ = F // NQ         # 1408 F-columns per core
KFQ = FQ // P        # 11 f-tiles per core
NFB = 3              # phase-1 weight DMA blocks per (expert, matrix)
FBW = 512            # f-columns per block (last block: 384 real + 128 pad)
FT_PER_FB = (4, 4, 3)
NDG = 8              # phase-2 d-groups (256 D-columns each)
DG = 2               # d-tiles per d-group

_COMPILED = {}       # (slot_sizes, chunks) -> nc
LAST_RESULT = None   # BassKernelResults of the most recent run (for test.py)


def _slot_chunks(s):
    """Split slot size s into <=512 even-sized chunks."""
    nch = -(-s // 512)
    base = s // nch
    sizes = [base] * nch
    for i in range(s - base * nch):
        sizes[i] += 1
    # make sizes even where possible (s is even)
    for i in range(0, len(sizes) - 1):
        if sizes[i] % 2:
            sizes[i] += 1
            sizes[i + 1] -= 1
    return sizes


def _build(slot_sizes):
    """Build + compile the SPMD Tile program for per-slot token counts."""
    import concourse.bass as bass  # noqa: F401
    import concourse.mybir as mybir
    import concourse.tile as tile
    from concourse import bacc

    f32 = mybir.dt.float32
    bf16 = mybir.dt.bfloat16
    mult = mybir.AluOpType.mult

    nslot = len(slot_sizes)
    slotmax = max(slot_sizes)
    cap = sum(slot_sizes)
    soffs = [sum(slot_sizes[:j]) for j in range(nslot)]
    chunks = [_slot_chunks(s) for s in slot_sizes]
    nch_max = max(len(c) for c in chunks)
    psum_bufs = 2 if nch_max <= 2 else 1

    nc = bacc.Bacc("TRN2", target_bir_lowering=False, debug=False,
                   enable_asserts=False, num_devices=E)

    # token tensor: per-(slot, chunk) contiguous [P, KD*cn] blocks packed
    # along the flat axis, so every segment DMA reads KD*cn-byte runs
    xt_d = nc.dram_tensor("xt", [P, KD * cap], bf16,
                          kind="ExternalInput").ap()
    wg_d = nc.dram_tensor("wg", [nslot, NFB, P, KD, FBW], bf16,
                          kind="ExternalInput").ap()
    wu_d = nc.dram_tensor("wu", [nslot, NFB, P, KD, FBW], bf16,
                          kind="ExternalInput").ap()
    # slot-0 first block again, f-slice-major: contiguous 4KB/partition
    # descriptors for the startup-critical fine-grained slices
    wgf_d = nc.dram_tensor("wgf", [4, P, KD, P], bf16,
                           kind="ExternalInput").ap()
    wuf_d = nc.dram_tensor("wuf", [4, P, KD, P], bf16,
                           kind="ExternalInput").ap()
    wd_d = nc.dram_tensor("wd", [nslot, NDG, P, KFQ, P * DG], bf16,
                          kind="ExternalInput").ap()
    wr_d = nc.dram_tensor("wrep", [P, cap], f32, kind="ExternalInput").ap()
    out_d = nc.dram_tensor("out_t", [D, cap], f32, kind="ExternalOutput").ap()
    scr_d = nc.dram_tensor("scr", [P, 512], f32).ap()   # warm-up sink

    with tile.TileContext(nc) as tc:
        with (
            tc.tile_pool(name="resident", bufs=1) as rpool,
            tc.tile_pool(name="xtp", bufs=2) as xtpool,
            tc.tile_pool(name="atp", bufs=2) as atpool,
            tc.tile_pool(name="w1", bufs=2) as w1pool,
            tc.tile_pool(name="wd2", bufs=8) as wd2pool,
            tc.tile_pool(name="outp", bufs=4) as outpool,
            tc.tile_pool(name="psum", bufs=psum_bufs, space="PSUM") as ppool,
        ):
            # ---- one SWDGE ring (gpsimd), loaded in consumption order:
            # slot-0 tokens + first weight block (f-major fine slices),
            # then per slot: w1 blocks / next-slot tokens / wd blocks.
            # sync (HWDGE) carries wrep, warm-up sink, output stores.
            xsegs = [None] * nslot

            def xseg_piece(j, c0j, cn, eng=None):
                f0 = (soffs[j] + c0j) * KD
                (eng or nc.gpsimd).dma_start(xsegs[j][:, :, c0j:c0j + cn],
                                             xt_d[:, f0:f0 + cn * KD])

            def xseg_dma(j):
                xsegs[j] = xtpool.tile([P, KD, slotmax], bf16, tag="xs",
                                       name=f"xseg_{j}")
                c0j = 0
                for cn in chunks[j]:
                    xseg_piece(j, c0j, cn)
                    c0j += cn

            w1tiles = [[None] * NFB for _ in range(nslot)]
            wgb0 = w1pool.tile([P, KD, FBW], bf16, tag="wg", name="wgb_0_0")
            wub0 = w1pool.tile([P, KD, FBW], bf16, tag="wu", name="wub_0_0")
            w1tiles[0][0] = (wgb0, wub0)
            xsegs[0] = xtpool.tile([P, KD, slotmax], bf16, tag="xs",
                                   name="xseg_0")
            xseg_piece(0, 0, chunks[0][0])
            c0rest = chunks[0][0]
            for s in range(4):
                sl = slice(s * P, (s + 1) * P)
                nc.gpsimd.dma_start(wgb0[:, :, sl], wgf_d[s])
                nc.gpsimd.dma_start(wub0[:, :, sl], wuf_d[s])
                if s == 0:
                    for cn in chunks[0][1:]:
                        xseg_piece(0, c0rest, cn)
                        c0rest += cn

            wrep = rpool.tile([P, cap], f32)
            nc.sync.dma_start(wrep[:], wr_d)

            # Warm-up: run throwaway matmuls on a zeroed tile while the
            # first weight/token DMAs are in flight, so the PE HAM
            # clock-gate opens (1.2 -> 2.4 GHz) before real work arrives.
            warm = rpool.tile([P, 512], bf16)
            nc.vector.memset(warm[:], 0.0)
            wps = ppool.tile([P, 512], f32, tag="g0", name="warm_ps")
            NWARM = 64
            for i in range(NWARM):
                nc.tensor.matmul(wps[:, :256], warm[:, :P], warm[:, :256],
                                 start=(i == 0), stop=(i == NWARM - 1))
            wout = rpool.tile([P, 512], f32)
            nc.vector.tensor_copy(out=wout[:], in_=wps[:])
            nc.sync.dma_start(scr_d[:], wout[:])

            at_tiles = [None] * nslot

            def phase1(j):
                jchunks = chunks[j]
                jstarts = [sum(jchunks[:i]) for i in range(len(jchunks))]
                at = atpool.tile([P, KFQ, slotmax], bf16, tag="at",
                                 name=f"at_{j}")
                at_tiles[j] = at

                # ---- phase 1: gT/uT = W.T @ xT, aT = silu(gT)*uT ----
                xseg = xsegs[j]
                for fb in range(NFB):
                    if w1tiles[j][fb] is None:
                        wgb = w1pool.tile([P, KD, FBW], bf16, tag="wg",
                                          name=f"wgb_{j}_{fb}")
                        wub = w1pool.tile([P, KD, FBW], bf16, tag="wu",
                                          name=f"wub_{j}_{fb}")
                        w1tiles[j][fb] = (wgb, wub)
                        nc.gpsimd.dma_start(wgb[:], wg_d[j, fb])
                        nc.gpsimd.dma_start(wub[:], wu_d[j, fb])
                    if fb == 1 and j + 1 < nslot:
                        xseg_dma(j + 1)
                    wgb, wub = w1tiles[j][fb]
                    for fs in range(FT_PER_FB[fb]):
                        ft = fb * 4 + fs
                        for ci, (c0, cn) in enumerate(zip(jstarts, jchunks)):
                            pg = ppool.tile([P, 512], f32, tag=f"g{ci}")
                            pu = ppool.tile([P, 512], f32, tag=f"u{ci}")
                            for ko in range(KD):
                                nc.tensor.matmul(
                                    pg[:, :cn],
                                    wgb[:, ko, fs * P:(fs + 1) * P],
                                    xseg[:, ko, c0:c0 + cn],
                                    start=(ko == 0), stop=(ko == KD - 1))
                            for ko in range(KD):
                                nc.tensor.matmul(
                                    pu[:, :cn],
                                    wub[:, ko, fs * P:(fs + 1) * P],
                                    xseg[:, ko, c0:c0 + cn],
                                    start=(ko == 0), stop=(ko == KD - 1))
                            a_sl = at[:, ft, c0:c0 + cn]
                            nc.scalar.activation(
                                a_sl, pg[:, :cn],
                                mybir.ActivationFunctionType.Silu)
                            nc.vector.tensor_tensor(a_sl, a_sl, pu[:, :cn],
                                                    mult)

            def phase2(j):
                # ---- phase 2: dT = Wd.T @ aT (partial over F-quarter),
                #      out = dT * w ----
                soff = soffs[j]
                jchunks = chunks[j]
                jstarts = [sum(jchunks[:i]) for i in range(len(jchunks))]
                at = at_tiles[j]
                wdbs = []
                for dg in range(NDG):
                    wdb = wd2pool.tile([P, KFQ, P * DG], bf16, tag="wd",
                                       name=f"wdb_{j}_{dg}")
                    nc.gpsimd.dma_start(wdb[:], wd_d[j, dg])
                    wdbs.append(wdb)
                # one pass per chunk, 2 PSUM groups per d-group, with
                # dg-parity tag rotation: a d-group's first matmul only
                # reuses a PSUM bank 4 d-groups back, so the combine
                # (DVE) latency never stalls the PE
                for ci, (c0, cn) in enumerate(zip(jstarts, jchunks)):
                    for dg in range(NDG):
                        wdb = wdbs[dg]
                        pds = [ppool.tile([P, 512], f32,
                                          tag=f"{'gu'[ds]}{dg % 2}",
                                          name=f"pd_{j}_{dg}_{ds}_{ci}")
                               for ds in range(DG)]
                        for fk in range(KFQ):
                            for ds in range(DG):
                                nc.tensor.matmul(
                                    pds[ds][:, :cn],
                                    wdb[:, fk, ds * P:(ds + 1) * P],
                                    at[:, fk, c0:c0 + cn],
                                    start=(fk == 0), stop=(fk == KFQ - 1))
                        for ds in range(DG):
                            dt_idx = dg * DG + ds
                            ot = outpool.tile([P, 512], f32, tag="ot")
                            nc.vector.tensor_tensor(
                                ot[:, :cn], pds[ds][:, :cn],
                                wrep[:, soff + c0:soff + c0 + cn], mult)
                            nc.sync.dma_start(
                                out_d[dt_idx * P:(dt_idx + 1) * P,
                                      soff + c0:soff + c0 + cn],
                                ot[:, :cn])

            # Software-pipeline the slots: each phase 2 runs one slot
            # behind its phase 1, so the trailing silu/mult of slot j's
            # last f-tile completes long before phase 2 reads it (no PE
            # stall on the DVE at phase boundaries).
            phase1(0)
            for j in range(1, nslot):
                phase1(j)
                phase2(j - 1)
            phase2(nslot - 1)

    nc.compile()
    return nc


def _swizzle_w1(wq):
    """[D, FQ] bf16 -> [NFB, P, KD, FBW] block-major (last block padded)."""
    wp = np.zeros((D, NFB * FBW), dtype=wq.dtype)
    wp[:, :FQ] = wq
    return np.ascontiguousarray(
        wp.reshape(KD, P, NFB, FBW).transpose(2, 1, 0, 3))


def _swizzle_wd(wq):
    """[FQ, D] bf16 -> [NDG, P, KFQ, P*DG] block-major."""
    return np.ascontiguousarray(
        wq.reshape(KFQ, P, NDG, P * DG).transpose(2, 1, 0, 3))


def _swizzle_w1_first(wq):
    """[D, FQ] bf16 -> [4, P, KD, P] f-slice-major (first 512 f-cols)."""
    return np.ascontiguousarray(
        wq[:, :FBW].reshape(KD, P, 4, P).transpose(2, 1, 0, 3))


def kernel(x, gate_tensor, Wg, Wu, Wd):
    global LAST_RESULT
    import ml_dtypes
    from concourse.bass_interp import get_hw_module
    from concourse.bass_utils import run_bass_kernel_spmd

    bf = ml_dtypes.bfloat16
    x = np.ascontiguousarray(np.asarray(x, dtype=np.float32))
    gate_tensor = np.asarray(gate_tensor, dtype=np.float32)
    Wg = np.asarray(Wg, dtype=np.float32)
    Wu = np.asarray(Wu, dtype=np.float32)
    Wd = np.asarray(Wd, dtype=np.float32)

    # ---- router (replicated; tiny: T*D*E flops) ----
    logits = x @ gate_tensor                      # [T, E] fp32
    m = logits.max(axis=-1, keepdims=True)
    p = np.exp(logits - m, dtype=np.float32)
    p /= p.sum(axis=-1, keepdims=True)
    topi = np.argsort(-p, axis=-1, kind="stable")[:, :TOPK]      # [T, K]
    topw = np.take_along_axis(p, topi, axis=-1)
    topw = topw / (topw.sum(axis=-1, keepdims=True) + 1e-20)

    idx = []          # tokens routed to each expert
    wts = []          # their combine weights
    for e in range(E):
        sel = (topi == e)                         # [T, K]; <=1 True per row
        idx.append(np.nonzero(sel.any(axis=-1))[0])
        wts.append(topw[sel].astype(np.float32))  # row-major == token order

    # ---- balanced expert grouping: sort by load desc, alternate ranks ----
    order = sorted(range(E), key=lambda e: -len(idx[e]))
    groups = [[order[2 * j + g] for j in range(NQ)] for g in range(2)]
    slot_sizes = tuple(
        (max(len(idx[groups[0][j]]), len(idx[groups[1][j]])) + 1) // 2 * 2
        for j in range(NQ))
    soffs = [sum(slot_sizes[:j]) for j in range(NQ)]
    cap = sum(slot_sizes)

    if slot_sizes not in _COMPILED:
        _COMPILED[slot_sizes] = _build(slot_sizes)
    nc = _COMPILED[slot_sizes]

    # ---- dispatch: per-core inputs (bf16, pre-swizzled) ----
    x_bf = x.astype(bf)
    Wg_bf = Wg.astype(bf)
    Wu_bf = Wu.astype(bf)
    Wd_bf = Wd.astype(bf)

    in_maps = []
    for g in range(2):
        # tokens + combine weights shared by the group's 4 cores;
        # xt is packed as contiguous [P, KD*cn] blocks per (slot, chunk)
        xt = np.zeros((P, KD * cap), dtype=bf)
        wr = np.zeros((P, cap), dtype=np.float32)
        for j in range(NQ):
            e = groups[g][j]
            n = len(idx[e])
            xg = x_bf[idx[e]]                      # [n, D]
            wr[:, soffs[j]:soffs[j] + n] = wts[e][None, :]
            c0 = 0
            for cn in _slot_chunks(slot_sizes[j]):
                m = max(0, min(cn, n - c0))
                blk = np.zeros((P, KD, cn), dtype=bf)
                if m:
                    blk[:, :, :m] = (
                        xg[c0:c0 + m].T.reshape(KD, P, m).transpose(1, 0, 2))
                off = (soffs[j] + c0) * KD
                xt[:, off:off + KD * cn] = blk.reshape(P, KD * cn)
                c0 += cn
        for q in range(NQ):
            fsl = slice(q * FQ, (q + 1) * FQ)
            wg = np.stack([_swizzle_w1(Wg_bf[groups[g][j]][:, fsl])
                           for j in range(NQ)])
            wu = np.stack([_swizzle_w1(Wu_bf[groups[g][j]][:, fsl])
                           for j in range(NQ)])
            wd = np.stack([_swizzle_wd(Wd_bf[groups[g][j]][fsl, :])
                           for j in range(NQ)])
            wgf = _swizzle_w1_first(Wg_bf[groups[g][0]][:, fsl])
            wuf = _swizzle_w1_first(Wu_bf[groups[g][0]][:, fsl])
            in_maps.append({"xt": xt, "wg": wg, "wu": wu, "wd": wd,
                            "wgf": wgf, "wuf": wuf, "wrep": wr})

    trace = bool(int(os.environ.get("KERNEL_TRACE", "0")))
    old_m = nc.m
    nc.m = get_hw_module(nc.m)
    try:
        try:
            res = run_bass_kernel_spmd(nc, in_maps, core_ids=list(range(E)),
                                       trace=trace)
        except (ImportError, ModuleNotFoundError):
            # tracing requested (e.g. BASS_TRACE in the env) but this image
            # lacks the axon NTFF profile hook -- rerun without tracing
            os.environ["BASS_NEVER_TRACE"] = "1"
            res = run_bass_kernel_spmd(nc, in_maps, core_ids=list(range(E)),
                                       trace=False)
    finally:
        nc.m = old_m
    LAST_RESULT = res

    # ---- combine: tp_reduce over the 4 quarters, then scatter-add ----
    out = np.zeros((T, D), dtype=np.float32)
    for g in range(2):
        acc = res.results[g * NQ]["out_t"].astype(np.float64)
        for q in range(1, NQ):
            acc += res.results[g * NQ + q]["out_t"]
        acc = acc.astype(np.float32)
        for j in range(NQ):
            e = groups[g][j]
            n = len(idx[e])
            out[idx[e]] += acc[:, soffs[j]:soffs[j] + n].T
    return out


# revision 29
# speedup vs baseline: 1.0591x; 1.0201x over previous
"""BlockSparseMLP (MoE top-2 routing, 8 experts) — Trainium2 Bass kernel.

Strategy v2 (expert-group tensor-parallel, "TP-4"):

The 8 experts are split into 2 groups of 4, chosen to balance total token
load across groups (sorted-alternating assignment).  Each group owns 4
cores; core (g, q) holds the q-th F-quarter (1408 = 11 f-tiles) of all 4
experts in group g and computes, for every token routed to those experts,
the gated-MLP over its quarter:

   gT = Wg[e][:, q].T @ xT      (bf16 matmuls, fp32 PSUM)
   uT = Wu[e][:, q].T @ xT
   aT = silu(gT) * uT
   dT_partial = Wd[e][q, :].T @ aT      (partial over the F-quarter)
   outT += dT_partial * w_e             (combine weight folded on-chip)

Host sums the 4 partial outputs of each group (the tp_reduce) and
scatter-adds into the final [T, D] output.

Why: with one-expert-per-core SPMD, every core pays the *maximum* expert
load (545 tokens for seed 0).  With 4-way F-parallelism over expert
groups, a core processes its group's token total over a quarter of F;
balanced groups bring the per-core work to ~2064 token-quarters = 516
token-equivalents, recovering the load-imbalance loss and cutting the
instruction count (11 f-tiles per phase-1 pass instead of 44).

All inputs are cast to bf16 on the host and uploaded as bf16 — HBM
traffic halves vs fp32+SWDGE-cast, which removes the startup weight-
stream starvation the fp32 version showed in traces.
"""

import os

import numpy as np

T, D, F, E, TOPK = 2048, 2048, 5632, 8, 2
P = 128
KD = D // P          # 16 k-subtiles over D
NQ = 4               # cores per expert group (F quartering)
FQ = F // NQ         # 1408 F-columns per core
KFQ = FQ // P        # 11 f-tiles per core
NFB = 3              # phase-1 weight DMA blocks per (expert, matrix)
FBW = 512            # f-columns per block (last block: 384 real + 128 pad)
FT_PER_FB = (4, 4, 3)
NDG = 8              # phase-2 d-groups (256 D-columns each)
DG = 2               # d-tiles per d-group

_COMPILED = {}       # (slot_sizes, chunks) -> nc
LAST_RESULT = None   # BassKernelResults of the most recent run (for test.py)


def _slot_chunks(s):
    """Split slot size s into <=512 even-sized chunks."""
    nch = -(-s // 512)
    base = s // nch
    sizes = [base] * nch
    for i in range(s - base * nch):
        sizes[i] += 1
    # make sizes even where possible (s is even)
    for i in range(0, len(sizes) - 1):
        if sizes[i] % 2:
            sizes[i] += 1
            sizes[i + 1] -= 1
    return sizes


def _build(slot_sizes):
    """Build + compile the SPMD Tile program for per-slot token counts."""
    import concourse.bass as bass  # noqa: F401
    import concourse.mybir as mybir
    import concourse.tile as tile
    from concourse import bacc

    f32 = mybir.dt.float32
    bf16 = mybir.dt.bfloat16
    mult = mybir.AluOpType.mult

    nslot = len(slot_sizes)
    slotmax = max(slot_sizes)
    cap = sum(slot_sizes)
    soffs = [sum(slot_sizes[:j]) for j in range(nslot)]
    chunks = [_slot_chunks(s) for s in slot_sizes]
    nch_max = max(len(c) for c in chunks)
    psum_bufs = 2 if nch_max <= 2 else 1

    nc = bacc.Bacc("TRN2", target_bir_lowering=False, debug=False,
                   enable_asserts=False, num_devices=E)

    # token tensor: per-(slot, chunk) contiguous [P, KD*cn] blocks packed
    # along the flat axis, so every segment DMA reads KD*cn-byte runs
    xt_d = nc.dram_tensor("xt", [P, KD * cap], bf16,
                          kind="ExternalInput").ap()
    wg_d = nc.dram_tensor("wg", [nslot, NFB, P, KD, FBW], bf16,
                          kind="ExternalInput").ap()
    wu_d = nc.dram_tensor("wu", [nslot, NFB, P, KD, FBW], bf16,
                          kind="ExternalInput").ap()
    # slot-0 first block again, f-slice-major: contiguous 4KB/partition
    # descriptors for the startup-critical fine-grained slices
    wgf_d = nc.dram_tensor("wgf", [4, P, KD, P], bf16,
                           kind="ExternalInput").ap()
    wuf_d = nc.dram_tensor("wuf", [4, P, KD, P], bf16,
                           kind="ExternalInput").ap()
    wd_d = nc.dram_tensor("wd", [nslot, NDG, P, KFQ, P * DG], bf16,
                          kind="ExternalInput").ap()
    wr_d = nc.dram_tensor("wrep", [P, cap], f32, kind="ExternalInput").ap()
    out_d = nc.dram_tensor("out_t", [D, cap], f32, kind="ExternalOutput").ap()
    scr_d = nc.dram_tensor("scr", [P, 512], f32).ap()   # warm-up sink

    with tile.TileContext(nc) as tc:
        with (
            tc.tile_pool(name="resident", bufs=1) as rpool,
            tc.tile_pool(name="xtp", bufs=2) as xtpool,
            tc.tile_pool(name="atp", bufs=2) as atpool,
            tc.tile_pool(name="w1", bufs=2) as w1pool,
            tc.tile_pool(name="wd2", bufs=8) as wd2pool,
            tc.tile_pool(name="outp", bufs=8) as outpool,
            tc.tile_pool(name="psum", bufs=psum_bufs, space="PSUM") as ppool,
        ):
            # ---- one SWDGE ring (gpsimd), loaded in consumption order:
            # slot-0 tokens + first weight block (f-major fine slices),
            # then per slot: w1 blocks / next-slot tokens / wd blocks.
            # sync (HWDGE) carries wrep, warm-up sink, output stores.
            xsegs = [None] * nslot

            def xseg_piece(j, c0j, cn, eng=None):
                f0 = (soffs[j] + c0j) * KD
                (eng or nc.gpsimd).dma_start(xsegs[j][:, :, c0j:c0j + cn],
                                             xt_d[:, f0:f0 + cn * KD])

            def xseg_dma(j):
                xsegs[j] = xtpool.tile([P, KD, slotmax], bf16, tag="xs",
                                       name=f"xseg_{j}")
                c0j = 0
                for cn in chunks[j]:
                    xseg_piece(j, c0j, cn)
                    c0j += cn

            w1tiles = [[None] * NFB for _ in range(nslot)]
            wgb0 = w1pool.tile([P, KD, FBW], bf16, tag="wg", name="wgb_0_0")
            wub0 = w1pool.tile([P, KD, FBW], bf16, tag="wu", name="wub_0_0")
            w1tiles[0][0] = (wgb0, wub0)
            xsegs[0] = xtpool.tile([P, KD, slotmax], bf16, tag="xs",
                                   name="xseg_0")
            xseg_piece(0, 0, chunks[0][0])
            c0rest = chunks[0][0]
            for s in range(4):
                sl = slice(s * P, (s + 1) * P)
                nc.gpsimd.dma_start(wgb0[:, :, sl], wgf_d[s])
                nc.gpsimd.dma_start(wub0[:, :, sl], wuf_d[s])
                if s == 0:
                    for cn in chunks[0][1:]:
                        xseg_piece(0, c0rest, cn)
                        c0rest += cn

            wrep = rpool.tile([P, cap], f32)
            nc.sync.dma_start(wrep[:], wr_d)

            # Warm-up: run throwaway matmuls on a zeroed tile while the
            # first weight/token DMAs are in flight, so the PE HAM
            # clock-gate opens (1.2 -> 2.4 GHz) before real work arrives.
            warm = rpool.tile([P, 512], bf16)
            nc.vector.memset(warm[:], 0.0)
            wps = ppool.tile([P, 512], f32, tag="g0", name="warm_ps")
            NWARM = 64
            for i in range(NWARM):
                nc.tensor.matmul(wps[:, :256], warm[:, :P], warm[:, :256],
                                 start=(i == 0), stop=(i == NWARM - 1))
            wout = rpool.tile([P, 512], f32)
            nc.vector.tensor_copy(out=wout[:], in_=wps[:])
            nc.sync.dma_start(scr_d[:], wout[:])

            at_tiles = [None] * nslot

            def phase1(j):
                jchunks = chunks[j]
                jstarts = [sum(jchunks[:i]) for i in range(len(jchunks))]
                at = atpool.tile([P, KFQ, slotmax], bf16, tag="at",
                                 name=f"at_{j}")
                at_tiles[j] = at

                # ---- phase 1: gT/uT = W.T @ xT, aT = silu(gT)*uT ----
                xseg = xsegs[j]
                for fb in range(NFB):
                    if w1tiles[j][fb] is None:
                        wgb = w1pool.tile([P, KD, FBW], bf16, tag="wg",
                                          name=f"wgb_{j}_{fb}")
                        wub = w1pool.tile([P, KD, FBW], bf16, tag="wu",
                                          name=f"wub_{j}_{fb}")
                        w1tiles[j][fb] = (wgb, wub)
                        nc.gpsimd.dma_start(wgb[:], wg_d[j, fb])
                        nc.gpsimd.dma_start(wub[:], wu_d[j, fb])
                    if fb == 1 and j + 1 < nslot:
                        xseg_dma(j + 1)
                    wgb, wub = w1tiles[j][fb]
                    # slot-0 fb-0 runs chunk-outer so the second token
                    # chunk's DMA has until ~ft3 to land
                    if j == 0 and fb == 0:
                        fsci = [(fs, ci) for ci in range(len(jchunks))
                                for fs in range(FT_PER_FB[fb])]
                    else:
                        fsci = [(fs, ci) for fs in range(FT_PER_FB[fb])
                                for ci in range(len(jchunks))]
                    for fs, ci in fsci:
                        ft = fb * 4 + fs
                        c0, cn = jstarts[ci], jchunks[ci]
                        if True:
                            pg = ppool.tile([P, 512], f32, tag=f"g{ci}")
                            pu = ppool.tile([P, 512], f32, tag=f"u{ci}")
                            for ko in range(KD):
                                nc.tensor.matmul(
                                    pg[:, :cn],
                                    wgb[:, ko, fs * P:(fs + 1) * P],
                                    xseg[:, ko, c0:c0 + cn],
                                    start=(ko == 0), stop=(ko == KD - 1))
                            for ko in range(KD):
                                nc.tensor.matmul(
                                    pu[:, :cn],
                                    wub[:, ko, fs * P:(fs + 1) * P],
                                    xseg[:, ko, c0:c0 + cn],
                                    start=(ko == 0), stop=(ko == KD - 1))
                            a_sl = at[:, ft, c0:c0 + cn]
                            nc.scalar.activation(
                                a_sl, pg[:, :cn],
                                mybir.ActivationFunctionType.Silu)
                            nc.vector.tensor_tensor(a_sl, a_sl, pu[:, :cn],
                                                    mult)

            def phase2(j):
                # ---- phase 2: dT = Wd.T @ aT (partial over F-quarter),
                #      out = dT * w ----
                soff = soffs[j]
                jchunks = chunks[j]
                jstarts = [sum(jchunks[:i]) for i in range(len(jchunks))]
                at = at_tiles[j]
                wdbs = []
                for dg in range(NDG):
                    wdb = wd2pool.tile([P, KFQ, P * DG], bf16, tag="wd",
                                       name=f"wdb_{j}_{dg}")
                    nc.gpsimd.dma_start(wdb[:], wd_d[j, dg])
                    wdbs.append(wdb)
                # one pass per chunk, 2 PSUM groups per d-group, with
                # dg-parity tag rotation: a d-group's first matmul only
                # reuses a PSUM bank 4 d-groups back, so the combine
                # (DVE) latency never stalls the PE
                for ci, (c0, cn) in enumerate(zip(jstarts, jchunks)):
                    for dg in range(NDG):
                        wdb = wdbs[dg]
                        pds = [ppool.tile([P, 512], f32,
                                          tag=f"{'gu'[ds]}{dg % 2}",
                                          name=f"pd_{j}_{dg}_{ds}_{ci}")
                               for ds in range(DG)]
                        for fk in range(KFQ):
                            for ds in range(DG):
                                nc.tensor.matmul(
                                    pds[ds][:, :cn],
                                    wdb[:, fk, ds * P:(ds + 1) * P],
                                    at[:, fk, c0:c0 + cn],
                                    start=(fk == 0), stop=(fk == KFQ - 1))
                        for ds in range(DG):
                            dt_idx = dg * DG + ds
                            ot = outpool.tile([P, 512], f32, tag="ot")
                            nc.vector.tensor_tensor(
                                ot[:, :cn], pds[ds][:, :cn],
                                wrep[:, soff + c0:soff + c0 + cn], mult)
                            nc.sync.dma_start(
                                out_d[dt_idx * P:(dt_idx + 1) * P,
                                      soff + c0:soff + c0 + cn],
                                ot[:, :cn])

            # Software-pipeline the slots: each phase 2 runs one slot
            # behind its phase 1, so the trailing silu/mult of slot j's
            # last f-tile completes long before phase 2 reads it (no PE
            # stall on the DVE at phase boundaries).
            phase1(0)
            for j in range(1, nslot):
                phase1(j)
                phase2(j - 1)
            phase2(nslot - 1)

    nc.compile()
    return nc


def _swizzle_w1(wq):
    """[D, FQ] bf16 -> [NFB, P, KD, FBW] block-major (last block padded)."""
    wp = np.zeros((D, NFB * FBW), dtype=wq.dtype)
    wp[:, :FQ] = wq
    return np.ascontiguousarray(
        wp.reshape(KD, P, NFB, FBW).transpose(2, 1, 0, 3))


def _swizzle_wd(wq):
    """[FQ, D] bf16 -> [NDG, P, KFQ, P*DG] block-major."""
    return np.ascontiguousarray(
        wq.reshape(KFQ, P, NDG, P * DG).transpose(2, 1, 0, 3))


def _swizzle_w1_first(wq):
    """[D, FQ] bf16 -> [4, P, KD, P] f-slice-major (first 512 f-cols)."""
    return np.ascontiguousarray(
        wq[:, :FBW].reshape(KD, P, 4, P).transpose(2, 1, 0, 3))


def kernel(x, gate_tensor, Wg, Wu, Wd):
    global LAST_RESULT
    import ml_dtypes
    from concourse.bass_interp import get_hw_module
    from concourse.bass_utils import run_bass_kernel_spmd

    bf = ml_dtypes.bfloat16
    x = np.ascontiguousarray(np.asarray(x, dtype=np.float32))
    gate_tensor = np.asarray(gate_tensor, dtype=np.float32)
    Wg = np.asarray(Wg, dtype=np.float32)
    Wu = np.asarray(Wu, dtype=np.float32)
    Wd = np.asarray(Wd, dtype=np.float32)

    # ---- router (replicated; tiny: T*D*E flops) ----
    logits = x @ gate_tensor                      # [T, E] fp32
    m = logits.max(axis=-1, keepdims=True)
    p = np.exp(logits - m, dtype=np.float32)
    p /= p.sum(axis=-1, keepdims=True)
    topi = np.argsort(-p, axis=-1, kind="stable")[:, :TOPK]      # [T, K]
    topw = np.take_along_axis(p, topi, axis=-1)
    topw = topw / (topw.sum(axis=-1, keepdims=True) + 1e-20)

    idx = []          # tokens routed to each expert
    wts = []          # their combine weights
    for e in range(E):
        sel = (topi == e)                         # [T, K]; <=1 True per row
        idx.append(np.nonzero(sel.any(axis=-1))[0])
        wts.append(topw[sel].astype(np.float32))  # row-major == token order

    # ---- balanced expert grouping: sort by load desc, alternate ranks ----
    order = sorted(range(E), key=lambda e: -len(idx[e]))
    groups = [[order[2 * j + g] for j in range(NQ)] for g in range(2)]
    slot_sizes = tuple(
        (max(len(idx[groups[0][j]]), len(idx[groups[1][j]])) + 1) // 2 * 2
        for j in range(NQ))
    soffs = [sum(slot_sizes[:j]) for j in range(NQ)]
    cap = sum(slot_sizes)

    if slot_sizes not in _COMPILED:
        _COMPILED[slot_sizes] = _build(slot_sizes)
    nc = _COMPILED[slot_sizes]

    # ---- dispatch: per-core inputs (bf16, pre-swizzled) ----
    x_bf = x.astype(bf)
    Wg_bf = Wg.astype(bf)
    Wu_bf = Wu.astype(bf)
    Wd_bf = Wd.astype(bf)

    in_maps = []
    for g in range(2):
        # tokens + combine weights shared by the group's 4 cores;
        # xt is packed as contiguous [P, KD*cn] blocks per (slot, chunk)
        xt = np.zeros((P, KD * cap), dtype=bf)
        wr = np.zeros((P, cap), dtype=np.float32)
        for j in range(NQ):
            e = groups[g][j]
            n = len(idx[e])
            xg = x_bf[idx[e]]                      # [n, D]
            wr[:, soffs[j]:soffs[j] + n] = wts[e][None, :]
            c0 = 0
            for cn in _slot_chunks(slot_sizes[j]):
                m = max(0, min(cn, n - c0))
                blk = np.zeros((P, KD, cn), dtype=bf)
                if m:
                    blk[:, :, :m] = (
                        xg[c0:c0 + m].T.reshape(KD, P, m).transpose(1, 0, 2))
                off = (soffs[j] + c0) * KD
                xt[:, off:off + KD * cn] = blk.reshape(P, KD * cn)
                c0 += cn
        for q in range(NQ):
            fsl = slice(q * FQ, (q + 1) * FQ)
            wg = np.stack([_swizzle_w1(Wg_bf[groups[g][j]][:, fsl])
                           for j in range(NQ)])
            wu = np.stack([_swizzle_w1(Wu_bf[groups[g][j]][:, fsl])
                           for j in range(NQ)])
            wd = np.stack([_swizzle_wd(Wd_bf[groups[g][j]][fsl, :])
                           for j in range(NQ)])
            wgf = _swizzle_w1_first(Wg_bf[groups[g][0]][:, fsl])
            wuf = _swizzle_w1_first(Wu_bf[groups[g][0]][:, fsl])
            in_maps.append({"xt": xt, "wg": wg, "wu": wu, "wd": wd,
                            "wgf": wgf, "wuf": wuf, "wrep": wr})

    trace = bool(int(os.environ.get("KERNEL_TRACE", "0")))
    old_m = nc.m
    nc.m = get_hw_module(nc.m)
    try:
        try:
            res = run_bass_kernel_spmd(nc, in_maps, core_ids=list(range(E)),
                                       trace=trace)
        except (ImportError, ModuleNotFoundError):
            # tracing requested (e.g. BASS_TRACE in the env) but this image
            # lacks the axon NTFF profile hook -- rerun without tracing
            os.environ["BASS_NEVER_TRACE"] = "1"
            res = run_bass_kernel_spmd(nc, in_maps, core_ids=list(range(E)),
                                       trace=False)
    finally:
        nc.m = old_m
    LAST_RESULT = res

    # ---- combine: tp_reduce over the 4 quarters, then scatter-add ----
    out = np.zeros((T, D), dtype=np.float32)
    for g in range(2):
        acc = res.results[g * NQ]["out_t"].astype(np.float64)
        for q in range(1, NQ):
            acc += res.results[g * NQ + q]["out_t"]
        acc = acc.astype(np.float32)
        for j in range(NQ):
            e = groups[g][j]
            n = len(idx[e])
            out[idx[e]] += acc[:, soffs[j]:soffs[j] + n].T
    return out
